# revision 24
# baseline (speedup 1.0000x reference)
"""Diagonal(causal)-masked multi-head attention block on 8 trn2 NeuronCores.

Sharding: tensor-parallel over heads (16 heads -> 2 per core); every core
processes both batch elements for its 2 heads.  q/k/v projections are
column-sharded, out-projection is row-sharded; the partial outputs are
summed on the host (+ output bias).

Per-core dataflow (b in {0,1}, local heads h in {0,1}):
  P1  qT/kT/vT[dim, token] = W.T-chunks @ xT       (f32r, N=512 matmuls)
  P2  v_nat[token, dim] via PE transpose (bf16), ones-column appended
      (ones column carries the key-padding mask -> softmax denominator)
  P3  scores.T[k,q] per (k-tile 128, q-tile 512); both heads packed in one
      [128,1024] PSUM pair-tile (row-group-concurrent matmuls, K=64);
      exp on ACT (no max subtraction -- scores are O(1) by construction);
      causal zeroing via gpsimd affine_select on the bf16 E tile;
      PV: ctx.T[65, q] += v'[k,65].T @ E[k,q]  (bf16), row 64 = denom;
      normalize via K=2 broadcast matmul + DVE muls -> ctxT[128e, t] bf16
  P4  out_partial[t, :] = ctxT-chunk.T @ WoT  (bf16, row-sharded)
"""

import numpy as np
import ml_dtypes

import concourse.bacc as bacc
import concourse.mybir as mybir
import concourse.tile as tile
from concourse.bass_utils import run_bass_kernel_spmd

B = 2
S = 2048
E = 1024
H = 16
DH = 64
SCALE = DH**-0.5
NCORES = 8
HPC = H // NCORES  # heads per core (2)
EC = HPC * DH  # embed slice per core (128)

F32 = mybir.dt.float32
F32R = mybir.dt.float32r
BF16 = mybir.dt.bfloat16

KT = S // 128  # 16 k-tiles per sequence
QT = S // 512  # 4 q-tiles per sequence
IT = E // 128  # 8 contraction chunks for projections


def _build_program():
    nc = bacc.Bacc("TRN2", target_bir_lowering=False, debug=False,
                   num_devices=NCORES)

    xT = nc.dram_tensor("xT", [B, 128, QT, IT, 512], F32R,
                        kind="ExternalInput")
    wq = nc.dram_tensor("wq", [128, IT, 128], F32R, kind="ExternalInput")
    wk = nc.dram_tensor("wk", [128, IT, 128], F32R, kind="ExternalInput")
    wv = nc.dram_tensor("wv", [128, IT, 128], F32R, kind="ExternalInput")
    bqv = nc.dram_tensor("bqv", [128, 3], F32, kind="ExternalInput")
    wo = nc.dram_tensor("wo", [128, E], BF16, kind="ExternalInput")
    km = nc.dram_tensor("km", [128, B * KT], F32, kind="ExternalInput")
    ones = nc.dram_tensor("ones", [128, 128], F32R, kind="ExternalInput")
    opart = nc.dram_tensor("opart", [B, S, E], F32, kind="ExternalOutput")

    with tile.TileContext(nc) as tc:
        with (
            tc.tile_pool(name="const", bufs=1) as const,
            tc.tile_pool(name="xp", bufs=3) as xp,
            tc.tile_pool(name="qk", bufs=2) as qk,
            tc.tile_pool(name="vt", bufs=2) as vtp,
            tc.tile_pool(name="vall", bufs=2) as vallp,
            tc.tile_pool(name="ep", bufs=3) as ep,
            tc.tile_pool(name="ctxsb", bufs=3) as ctxsbp,
            tc.tile_pool(name="dnp", bufs=2) as dnp,
            tc.tile_pool(name="outsb", bufs=2) as outsbp,
            tc.tile_pool(name="scps", bufs=2, space="PSUM") as scps,
            tc.tile_pool(name="ctxps", bufs=1, space="PSUM") as ctxps,
            tc.tile_pool(name="auxps", bufs=2, space="PSUM") as auxps,
        ):
            # x segments: (chunk index, col offset within chunk, width);
            # chunk 0 split in half so the first projection starts sooner
            SEGS = [(0, 0, 256), (0, 256, 256), (1, 0, 512), (2, 0, 512),
                    (3, 0, 512)]

            def load_x_seg(b, seg):
                jt, c0, w = seg
                xc = xp.tile([128, IT, w], F32R,
                             tag="x" if w == 512 else "xh",
                             bufs=3 if w == 512 else 2)
                nc.sync.dma_start(out=xc, in_=xT[b, :, jt, :, c0:c0 + w])
                return xc

            # ---- constants (first x segment first: critical path) ----
            x_pre = [load_x_seg(0, SEGS[0])]
            wq_sb = const.tile([128, IT, 128], F32R, tag="wq")
            nc.sync.dma_start(out=wq_sb, in_=wq[:, :, :])
            x_pre.append(load_x_seg(0, SEGS[1]))
            wk_sb = const.tile([128, IT, 128], F32R, tag="wk")
            wv_sb = const.tile([128, IT, 128], F32R, tag="wv")
            nc.sync.dma_start(out=wk_sb, in_=wk[:, :, :])
            nc.sync.dma_start(out=wv_sb, in_=wv[:, :, :])
            bqv_sb = const.tile([128, 3], F32, tag="bqv")
            nc.sync.dma_start(out=bqv_sb, in_=bqv[:, :])
            wo_sb = const.tile([128, E], BF16, tag="wo")
            nc.sync.dma_start(out=wo_sb, in_=wo[:, :])
            ones_sb = const.tile([128, 128], F32R, tag="ones")
            nc.sync.dma_start(out=ones_sb, in_=ones[:, :])
            km_sb = const.tile([128, B * KT], F32, tag="km")
            nc.sync.dma_start(out=km_sb, in_=km[:, :])
            ident = const.tile([128, 128], BF16, tag="ident")
            nc.gpsimd.memset(ident[:], 0.0)
            nc.gpsimd.affine_select(
                out=ident[:], in_=ident[:],
                compare_op=mybir.AluOpType.not_equal, fill=1.0,
                base=0, pattern=[[-1, 128]], channel_multiplier=1,
            )
            # causal triangle for the 128x128 diagonal boundary block:
            # tri[k, q] = 1.0 where q >= k else 0.0
            tri = const.tile([128, 128], BF16, tag="tri")
            nc.gpsimd.memset(tri[:], 1.0)
            nc.gpsimd.affine_select(
                out=tri[:], in_=tri[:],
                compare_op=mybir.AluOpType.is_ge, fill=0.0,
                base=0, pattern=[[1, 128]], channel_multiplier=-1,
            )

            for b in range(B):
                # ---- P1: load xT(b) segments, project q/k/v ----
                if b == 0:
                    x_segs = x_pre + [load_x_seg(0, s) for s in SEGS[2:]]
                else:
                    x_segs = [load_x_seg(b, s) for s in SEGS]

                qT = qk.tile([128, S], F32R, tag="qT")
                kT = qk.tile([128, S], F32R, tag="kT")
                vT = vtp.tile([128, S], BF16, tag="vT")
                for si, (jt, c0, w) in enumerate(SEGS):
                    col = jt * 512 + c0
                    for (w_sb, dst, bcol) in ((wq_sb, qT, 0), (wk_sb, kT, 1),
                                              (wv_sb, vT, 2)):
                        ps = auxps.tile([128, 512], F32, tag="aux")
                        for a in range(IT):
                            nc.tensor.matmul(
                                ps[:, 0:w],
                                w_sb[:, a, :],
                                x_segs[si][:, a, :],
                                start=(a == 0), stop=(a == IT - 1),
                            )
                        nc.vector.tensor_scalar_add(
                            out=dst[:, col:col + w],
                            in0=ps[:, 0:w],
                            scalar1=bqv_sb[:, bcol:bcol + 1],
                        )

                # ---- P2: v -> natural layout tiles with ones column ----
                # v_all[:, h, ik, 0:64] = v_nat rows for head h, k-tile ik
                # v_all[:, h, ik, 64]   = key-padding mask (all-ones normally)
                v_all = vallp.tile([128, HPC, KT, 65], BF16, tag="vall")
                for ik in range(KT):
                    tp = auxps.tile([128, 128], BF16, tag="aux")
                    nc.tensor.transpose(
                        tp[:], vT[:, ik * 128:(ik + 1) * 128], ident[:])
                    col = b * KT + ik
                    for h in range(HPC):
                        nc.vector.tensor_scalar_mul(
                            out=v_all[:, h, ik, 0:64],
                            in0=tp[:, h * 64:(h + 1) * 64],
                            scalar1=km_sb[:, col:col + 1],
                        )
                        nc.vector.tensor_copy(
                            out=v_all[:, h, ik, 64:65],
                            in_=km_sb[:, col:col + 1],
                        )

                # ---- P3: attention (out-projection interleaved per jq) ----
                for jq in range(QT):
                    nk = 4 * (jq + 1)  # causal: k-tiles 0..nk-1 only
                    ctx = ctxps.tile([65, 1024], F32, tag="ctx")
                    ctxT = ctxsbp.tile([128, 512], BF16, tag="ctxT")
                    qsl = slice(jq * 512, (jq + 1) * 512)
                    for ik in range(nk):
                        ksl = slice(ik * 128, (ik + 1) * 128)
                        m = ik - 4 * jq  # >=0 on the diagonal band
                        sc = scps.tile([128, 1024], F32, tag="sc")
                        for h in range(HPC):
                            hsl = slice(h * 64, (h + 1) * 64)
                            nc.tensor.matmul(
                                sc[:, h * 512:(h + 1) * 512],
                                kT[hsl, ksl],
                                qT[hsl, qsl],
                                start=True, stop=True,
                            )
                        if m > 0:
                            # fully-masked column block: q < k everywhere
                            nc.vector.memset(
                                sc[:].rearrange("p (h q) -> p h q",
                                                h=2)[:, :, 0:128 * m],
                                -100.0,
                            )
                        e = ep.tile([128, 1024], BF16, tag="e")
                        nc.scalar.activation(
                            out=e[:], in_=sc[:],
                            func=mybir.ActivationFunctionType.Exp,
                        )
                        if m >= 0:
                            # triangle-mask the 128-wide boundary block
                            for h in range(HPC):
                                bsl = slice(h * 512 + 128 * m,
                                            h * 512 + 128 * m + 128)
                                nc.vector.tensor_mul(
                                    out=e[:, bsl], in0=e[:, bsl], in1=tri[:])
                        for h in range(HPC):
                            nc.tensor.matmul(
                                ctx[:, h * 512:(h + 1) * 512],
                                v_all[:, h, ik, :],
                                e[:, h * 512:(h + 1) * 512],
                                start=(ik == 0), stop=(ik == nk - 1),
                            )
                    # evacuate ctx psum in one copy (row 64 = denominators),
                    # then broadcast -> reciprocal -> normalize from SBUF
                    ctxu = dnp.tile([65, 1024], F32R, tag="ctxu")
                    nc.scalar.copy(out=ctxu[:], in_=ctx[:])
                    for h in range(HPC):
                        rb = auxps.tile([128, 512], F32, tag="aux")
                        nc.tensor.matmul(
                            rb[:], ones_sb[64:65, :],
                            ctxu[64:65, h * 512:(h + 1) * 512],
                            start=True, stop=True)
                        rbr = dnp.tile([128, 512], F32, tag="rb")
                        nc.vector.reciprocal_approx_fast(out=rbr[:], in_=rb[:])
                        nc.vector.tensor_mul(
                            out=ctxT[h * 64:(h + 1) * 64, :],
                            in0=ctxu[0:64,
                                     h * 512:(h + 1) * 512].bitcast(F32),
                            in1=rbr[0:64, :],
                        )

                    # out-projection for this jq's 512 tokens (row-sharded)
                    for sub in range(4):
                        jt = jq * 4 + sub
                        tsl = slice(jt * 128, (jt + 1) * 128)
                        osb = outsbp.tile([128, E], F32, tag="osb")
                        po = scps.tile([128, 1024], F32, tag="sc")
                        for half in range(2):
                            osl = slice(half * 512, (half + 1) * 512)
                            nc.tensor.matmul(
                                po[:, osl], ctxT[:, sub * 128:(sub + 1) * 128],
                                wo_sb[:, osl], start=True, stop=True)
                            if half == 0:
                                nc.scalar.copy(out=osb[:, osl],
                                               in_=po[:, osl])
                            else:
                                nc.vector.tensor_copy(out=osb[:, osl],
                                                      in_=po[:, osl])
                        nc.gpsimd.dma_start(out=opart[b, tsl, :], in_=osb[:])

    nc.compile()
    return nc


_NC = None


def _get_program():
    global _NC
    if _NC is None:
        _NC = _build_program()
    return _NC


def _prep_in_maps(hidden_states, attention_mask, Wq, bq, Wk, bk, Wv, bv, Wo):
    hidden_states = np.asarray(hidden_states, dtype=np.float32)
    attention_mask = np.asarray(attention_mask)
    Wq = np.asarray(Wq, dtype=np.float32)
    Wk = np.asarray(Wk, dtype=np.float32)
    Wv = np.asarray(Wv, dtype=np.float32)
    Wo = np.asarray(Wo, dtype=np.float32)
    bq = np.asarray(bq, dtype=np.float32)
    bk = np.asarray(bk, dtype=np.float32)
    bv = np.asarray(bv, dtype=np.float32)

    # xT[b, p, a, t] = hidden[b, t, 128a+p]  (replicated to every core)
    xT = np.ascontiguousarray(
        hidden_states.transpose(0, 2, 1).reshape(B, IT, 128, QT, 512)
        .transpose(0, 2, 3, 1, 4))

    # km[p, b*KT + ik] = attention_mask[b, 128*ik + p]  (multiplicative 0/1)
    km = (attention_mask.astype(np.float32).reshape(B, KT, 128)
          .transpose(2, 0, 1).reshape(128, B * KT))
    km = np.ascontiguousarray(km)

    in_maps = []
    for c in range(NCORES):
        hs = slice(c * EC, (c + 1) * EC)

        def wprep(W, scale=1.0):
            wt = (scale * W[hs, :]).T  # [E, EC]
            return np.ascontiguousarray(
                wt.reshape(IT, 128, EC).transpose(1, 0, 2))

        bqv = np.stack([SCALE * bq[hs], bk[hs], bv[hs]], axis=1)
        in_maps.append({
            "xT": xT,
            "wq": wprep(Wq, SCALE),
            "wk": wprep(Wk),
            "wv": wprep(Wv),
            "bqv": np.ascontiguousarray(bqv),
            "wo": np.ascontiguousarray(Wo[:, hs].T).astype(ml_dtypes.bfloat16),
            "km": km,
            "ones": np.ones((128, 128), dtype=np.float32),
        })
    return in_maps


def kernel(hidden_states, attention_mask, Wq, bq, Wk, bk, Wv, bv, Wo, bo):
    in_maps = _prep_in_maps(hidden_states, attention_mask,
                            Wq, bq, Wk, bk, Wv, bv, Wo)
    bo = np.asarray(bo, dtype=np.float32)
    nc = _get_program()
    res = run_bass_kernel_spmd(nc, in_maps, core_ids=list(range(NCORES)))

    out = res.results[0]["opart"].astype(np.float64)
    for c in range(1, NCORES):
        out += res.results[c]["opart"]
    out += bo
    return out.astype(np.float32)


# revision 25
# speedup vs baseline: 1.6122x; 1.6122x over previous
"""Diagonal(causal)-masked multi-head attention block on 8 trn2 NeuronCores.

Sharding: tensor-parallel over heads (16 heads -> 2 per core); every core
processes both batch elements for its 2 heads.  q/k/v projections are
column-sharded, out-projection is row-sharded; the partial outputs are
summed on the host (+ output bias).

Per-core dataflow (b in {0,1}, local heads h in {0,1}):
  P1  qT/kT/vT[dim, token] = W.T-chunks @ xT       (f32r, N=512 matmuls)
  P2  v_nat[token, dim] via PE transpose (bf16), ones-column appended
      (ones column carries the key-padding mask -> softmax denominator)
  P3  scores.T[k,q] per (k-tile 128, q-tile 512); both heads packed in one
      [128,1024] PSUM pair-tile (row-group-concurrent matmuls, K=64);
      exp on ACT (no max subtraction -- scores are O(1) by construction);
      causal zeroing via gpsimd affine_select on the bf16 E tile;
      PV: ctx.T[65, q] += v'[k,65].T @ E[k,q]  (bf16), row 64 = denom;
      normalize via K=2 broadcast matmul + DVE muls -> ctxT[128e, t] bf16
  P4  out_partial[t, :] = ctxT-chunk.T @ WoT  (bf16, row-sharded)
"""

import numpy as np
import ml_dtypes

import concourse.bacc as bacc
import concourse.mybir as mybir
import concourse.tile as tile
from concourse.bass_utils import run_bass_kernel_spmd

B = 2
S = 2048
E = 1024
H = 16
DH = 64
SCALE = DH**-0.5
NCORES = 8
HPC = H // NCORES  # heads per core (2)
EC = HPC * DH  # embed slice per core (128)

F32 = mybir.dt.float32
F32R = mybir.dt.float32r
BF16 = mybir.dt.bfloat16

KT = S // 128  # 16 k-tiles per sequence
QT = S // 512  # 4 q-tiles per sequence
IT = E // 128  # 8 contraction chunks for projections


def _build_program():
    nc = bacc.Bacc("TRN2", target_bir_lowering=False, debug=False,
                   num_devices=NCORES)

    xT = nc.dram_tensor("xT", [B, 128, QT, IT, 512], F32R,
                        kind="ExternalInput")
    wq = nc.dram_tensor("wq", [128, IT, 128], F32R, kind="ExternalInput")
    wk = nc.dram_tensor("wk", [128, IT, 128], F32R, kind="ExternalInput")
    wv = nc.dram_tensor("wv", [128, IT, 128], F32R, kind="ExternalInput")
    bqv = nc.dram_tensor("bqv", [128, 3], F32, kind="ExternalInput")
    wo = nc.dram_tensor("wo", [128, E], BF16, kind="ExternalInput")
    km = nc.dram_tensor("km", [128, B * KT], F32, kind="ExternalInput")
    ones = nc.dram_tensor("ones", [128, 128], F32R, kind="ExternalInput")
    opart = nc.dram_tensor("opart", [B, S, E], F32, kind="ExternalOutput")

    with tile.TileContext(nc) as tc:
        with (
            tc.tile_pool(name="const", bufs=1) as const,
            tc.tile_pool(name="xp", bufs=3) as xp,
            tc.tile_pool(name="qk", bufs=2) as qk,
            tc.tile_pool(name="vt", bufs=2) as vtp,
            tc.tile_pool(name="vall", bufs=2) as vallp,
            tc.tile_pool(name="ep", bufs=3) as ep,
            tc.tile_pool(name="ctxsb", bufs=3) as ctxsbp,
            tc.tile_pool(name="dnp", bufs=2) as dnp,
            tc.tile_pool(name="outsb", bufs=2) as outsbp,
            tc.tile_pool(name="scps", bufs=2, space="PSUM") as scps,
            tc.tile_pool(name="ctxps", bufs=1, space="PSUM") as ctxps,
            tc.tile_pool(name="auxps", bufs=2, space="PSUM") as auxps,
        ):
            # x segments: (chunk index, col offset within chunk, width);
            # chunk 0 split in half so the first projection starts sooner
            SEGS = [(0, 0, 256), (0, 256, 256), (1, 0, 512), (2, 0, 512),
                    (3, 0, 512)]

            def load_x_seg(b, seg):
                jt, c0, w = seg
                xc = xp.tile([128, IT, w], F32R,
                             tag="x" if w == 512 else "xh",
                             bufs=3 if w == 512 else 2)
                nc.sync.dma_start(out=xc, in_=xT[b, :, jt, :, c0:c0 + w])
                return xc

            # ---- constants (first x segment first: critical path) ----
            x_pre = [load_x_seg(0, SEGS[0])]
            wq_sb = const.tile([128, IT, 128], F32R, tag="wq")
            nc.sync.dma_start(out=wq_sb, in_=wq[:, :, :])
            x_pre.append(load_x_seg(0, SEGS[1]))
            wk_sb = const.tile([128, IT, 128], F32R, tag="wk")
            wv_sb = const.tile([128, IT, 128], F32R, tag="wv")
            nc.sync.dma_start(out=wk_sb, in_=wk[:, :, :])
            nc.sync.dma_start(out=wv_sb, in_=wv[:, :, :])
            bqv_sb = const.tile([128, 3], F32, tag="bqv")
            nc.sync.dma_start(out=bqv_sb, in_=bqv[:, :])
            wo_sb = const.tile([128, E], BF16, tag="wo")
            nc.sync.dma_start(out=wo_sb, in_=wo[:, :])
            ones_sb = const.tile([128, 128], F32R, tag="ones")
            nc.sync.dma_start(out=ones_sb, in_=ones[:, :])
            km_sb = const.tile([128, B * KT], F32, tag="km")
            nc.sync.dma_start(out=km_sb, in_=km[:, :])
            ident = const.tile([128, 128], BF16, tag="ident")
            nc.gpsimd.memset(ident[:], 0.0)
            nc.gpsimd.affine_select(
                out=ident[:], in_=ident[:],
                compare_op=mybir.AluOpType.not_equal, fill=1.0,
                base=0, pattern=[[-1, 128]], channel_multiplier=1,
            )
            # causal triangle for the 128x128 diagonal boundary block:
            # tri[k, q] = 1.0 where q >= k else 0.0
            tri = const.tile([128, 128], BF16, tag="tri")
            nc.gpsimd.memset(tri[:], 1.0)
            nc.gpsimd.affine_select(
                out=tri[:], in_=tri[:],
                compare_op=mybir.AluOpType.is_ge, fill=0.0,
                base=0, pattern=[[1, 128]], channel_multiplier=-1,
            )

            for b in range(B):
                # ---- P1: load xT(b) segments, project q/k/v ----
                if b == 0:
                    x_segs = x_pre + [load_x_seg(0, s) for s in SEGS[2:]]
                else:
                    x_segs = [load_x_seg(b, s) for s in SEGS]

                qT = qk.tile([128, S], F32R, tag="qT")
                kT = qk.tile([128, S], F32R, tag="kT")
                vT = vtp.tile([128, S], BF16, tag="vT")
                for si, (jt, c0, w) in enumerate(SEGS):
                    col = jt * 512 + c0
                    for (w_sb, dst, bcol) in ((wq_sb, qT, 0), (wk_sb, kT, 1),
                                              (wv_sb, vT, 2)):
                        ps = auxps.tile([128, 512], F32, tag="aux")
                        for a in range(IT):
                            nc.tensor.matmul(
                                ps[:, 0:w],
                                w_sb[:, a, :],
                                x_segs[si][:, a, :],
                                start=(a == 0), stop=(a == IT - 1),
                            )
                        nc.vector.tensor_scalar_add(
                            out=dst[:, col:col + w],
                            in0=ps[:, 0:w],
                            scalar1=bqv_sb[:, bcol:bcol + 1],
                        )

                # ---- P2: v -> natural layout tiles with ones column ----
                # v_all[:, h, ik, 0:64] = v_nat rows for head h, k-tile ik
                # v_all[:, h, ik, 64]   = key-padding mask (all-ones normally)
                v_all = vallp.tile([128, HPC, KT, 65], BF16, tag="vall")
                for ik in range(KT):
                    tp = auxps.tile([128, 128], BF16, tag="aux")
                    nc.tensor.transpose(
                        tp[:], vT[:, ik * 128:(ik + 1) * 128], ident[:])
                    col = b * KT + ik
                    for h in range(HPC):
                        nc.vector.tensor_scalar_mul(
                            out=v_all[:, h, ik, 0:64],
                            in0=tp[:, h * 64:(h + 1) * 64],
                            scalar1=km_sb[:, col:col + 1],
                        )
                        nc.vector.tensor_copy(
                            out=v_all[:, h, ik, 64:65],
                            in_=km_sb[:, col:col + 1],
                        )

                # ---- P3: attention (out-projection interleaved per jq) ----
                for jq in range(QT):
                    nk = 4 * (jq + 1)  # causal: k-tiles 0..nk-1 only
                    ctx = ctxps.tile([65, 1024], F32, tag="ctx")
                    ctxT = ctxsbp.tile([128, 512], BF16, tag="ctxT")
                    qsl = slice(jq * 512, (jq + 1) * 512)
                    for ik in range(nk):
                        ksl = slice(ik * 128, (ik + 1) * 128)
                        m = ik - 4 * jq  # >=0 on the diagonal band
                        sc = scps.tile([128, 1024], F32, tag="sc")
                        for h in range(HPC):
                            hsl = slice(h * 64, (h + 1) * 64)
                            nc.tensor.matmul(
                                sc[:, h * 512:(h + 1) * 512],
                                kT[hsl, ksl],
                                qT[hsl, qsl],
                                start=True, stop=True,
                            )
                        if m > 0:
                            # fully-masked column block: q < k everywhere
                            nc.vector.memset(
                                sc[:].rearrange("p (h q) -> p h q",
                                                h=2)[:, :, 0:128 * m],
                                -100.0,
                            )
                        e = ep.tile([128, 1024], BF16, tag="e")
                        nc.scalar.activation(
                            out=e[:], in_=sc[:],
                            func=mybir.ActivationFunctionType.Exp,
                        )
                        if m >= 0:
                            # triangle-mask the 128-wide boundary block
                            for h in range(HPC):
                                bsl = slice(h * 512 + 128 * m,
                                            h * 512 + 128 * m + 128)
                                nc.vector.tensor_mul(
                                    out=e[:, bsl], in0=e[:, bsl], in1=tri[:])
                        for h in range(HPC):
                            nc.tensor.matmul(
                                ctx[:, h * 512:(h + 1) * 512],
                                v_all[:, h, ik, :],
                                e[:, h * 512:(h + 1) * 512],
                                start=(ik == 0), stop=(ik == nk - 1),
                            )
                    # evacuate ctx psum in one copy (row 64 = denominators),
                    # then broadcast -> reciprocal -> normalize from SBUF
                    ctxu = dnp.tile([65, 1024], F32R, tag="ctxu")
                    nc.scalar.copy(out=ctxu[:], in_=ctx[:])
                    for h in range(HPC):
                        rb = auxps.tile([128, 512], F32, tag="aux")
                        nc.tensor.matmul(
                            rb[:], ones_sb[64:65, :],
                            ctxu[64:65, h * 512:(h + 1) * 512],
                            start=True, stop=True)
                        rbr = dnp.tile([128, 512], F32, tag="rb")
                        nc.vector.reciprocal_approx_fast(out=rbr[:], in_=rb[:])
                        nc.vector.tensor_mul(
                            out=ctxT[h * 64:(h + 1) * 64, :],
                            in0=ctxu[0:64,
                                     h * 512:(h + 1) * 512].bitcast(F32),
                            in1=rbr[0:64, :],
                        )

                    # out-projection for this jq's 512 tokens (row-sharded)
                    for sub in range(4):
                        jt = jq * 4 + sub
                        tsl = slice(jt * 128, (jt + 1) * 128)
                        osb = outsbp.tile([128, E], F32, tag="osb")
                        for half in range(2):
                            osl = slice(half * 512, (half + 1) * 512)
                            po = auxps.tile([128, 512], F32, tag="aux")
                            nc.tensor.matmul(
                                po[:], ctxT[:, sub * 128:(sub + 1) * 128],
                                wo_sb[:, osl], start=True, stop=True)
                            if half == 0:
                                nc.scalar.copy(out=osb[:, osl], in_=po[:])
                            else:
                                nc.vector.tensor_copy(out=osb[:, osl],
                                                      in_=po[:])
                        nc.gpsimd.dma_start(out=opart[b, tsl, :], in_=osb[:])

    nc.compile()
    return nc


_NC = None


def _get_program():
    global _NC
    if _NC is None:
        _NC = _build_program()
    return _NC


def _prep_in_maps(hidden_states, attention_mask, Wq, bq, Wk, bk, Wv, bv, Wo):
    hidden_states = np.asarray(hidden_states, dtype=np.float32)
    attention_mask = np.asarray(attention_mask)
    Wq = np.asarray(Wq, dtype=np.float32)
    Wk = np.asarray(Wk, dtype=np.float32)
    Wv = np.asarray(Wv, dtype=np.float32)
    Wo = np.asarray(Wo, dtype=np.float32)
    bq = np.asarray(bq, dtype=np.float32)
    bk = np.asarray(bk, dtype=np.float32)
    bv = np.asarray(bv, dtype=np.float32)

    # xT[b, p, a, t] = hidden[b, t, 128a+p]  (replicated to every core)
    xT = np.ascontiguousarray(
        hidden_states.transpose(0, 2, 1).reshape(B, IT, 128, QT, 512)
        .transpose(0, 2, 3, 1, 4))

    # km[p, b*KT + ik] = attention_mask[b, 128*ik + p]  (multiplicative 0/1)
    km = (attention_mask.astype(np.float32).reshape(B, KT, 128)
          .transpose(2, 0, 1).reshape(128, B * KT))
    km = np.ascontiguousarray(km)

    in_maps = []
    for c in range(NCORES):
        hs = slice(c * EC, (c + 1) * EC)

        def wprep(W, scale=1.0):
            wt = (scale * W[hs, :]).T  # [E, EC]
            return np.ascontiguousarray(
                wt.reshape(IT, 128, EC).transpose(1, 0, 2))

        bqv = np.stack([SCALE * bq[hs], bk[hs], bv[hs]], axis=1)
        in_maps.append({
            "xT": xT,
            "wq": wprep(Wq, SCALE),
            "wk": wprep(Wk),
            "wv": wprep(Wv),
            "bqv": np.ascontiguousarray(bqv),
            "wo": np.ascontiguousarray(Wo[:, hs].T).astype(ml_dtypes.bfloat16),
            "km": km,
            "ones": np.ones((128, 128), dtype=np.float32),
        })
    return in_maps


def kernel(hidden_states, attention_mask, Wq, bq, Wk, bk, Wv, bv, Wo, bo):
    in_maps = _prep_in_maps(hidden_states, attention_mask,
                            Wq, bq, Wk, bk, Wv, bv, Wo)
    bo = np.asarray(bo, dtype=np.float32)
    nc = _get_program()
    res = run_bass_kernel_spmd(nc, in_maps, core_ids=list(range(NCORES)))

    out = res.results[0]["opart"].astype(np.float64)
    for c in range(1, NCORES):
        out += res.results[c]["opart"]
    out += bo
    return out.astype(np.float32)


# revision 28
# speedup vs baseline: 1.6288x; 1.0104x over previous
"""Diagonal(causal)-masked multi-head attention block on 8 trn2 NeuronCores.

Sharding: tensor-parallel over heads (16 heads -> 2 per core); every core
processes both batch elements for its 2 heads.  q/k/v projections are
column-sharded, out-projection is row-sharded; the partial outputs are
summed on the host (+ output bias).

Per-core dataflow (b in {0,1}, local heads h in {0,1}):
  P1  qT/kT/vT[dim, token] = W.T-chunks @ xT       (f32r, N=512 matmuls)
  P2  v_nat[token, dim] via PE transpose (bf16), ones-column appended
      (ones column carries the key-padding mask -> softmax denominator)
  P3  scores.T[k,q] per (k-tile 128, q-tile 512); both heads packed in one
      [128,1024] PSUM pair-tile (row-group-concurrent matmuls, K=64);
      exp on ACT (no max subtraction -- scores are O(1) by construction);
      causal zeroing via gpsimd affine_select on the bf16 E tile;
      PV: ctx.T[65, q] += v'[k,65].T @ E[k,q]  (bf16), row 64 = denom;
      normalize via K=2 broadcast matmul + DVE muls -> ctxT[128e, t] bf16
  P4  out_partial[t, :] = ctxT-chunk.T @ WoT  (bf16, row-sharded)
"""

import numpy as np
import ml_dtypes

import concourse.bass as bass
import concourse.bacc as bacc
import concourse.mybir as mybir
import concourse.tile as tile
from concourse.bass_utils import run_bass_kernel_spmd

B = 2
S = 2048
E = 1024
H = 16
DH = 64
SCALE = DH**-0.5
NCORES = 8
HPC = H // NCORES  # heads per core (2)
EC = HPC * DH  # embed slice per core (128)

F32 = mybir.dt.float32
F32R = mybir.dt.float32r
BF16 = mybir.dt.bfloat16

KT = S // 128  # 16 k-tiles per sequence
QT = S // 512  # 4 q-tiles per sequence
IT = E // 128  # 8 contraction chunks for projections


def _build_program():
    nc = bacc.Bacc("TRN2", target_bir_lowering=False, debug=False,
                   num_devices=NCORES)

    xT = nc.dram_tensor("xT", [B, 128, QT, IT, 512], F32R,
                        kind="ExternalInput")
    wq = nc.dram_tensor("wq", [128, IT, 128], F32R, kind="ExternalInput")
    wk = nc.dram_tensor("wk", [128, IT, 128], F32R, kind="ExternalInput")
    wv = nc.dram_tensor("wv", [128, IT, 128], F32R, kind="ExternalInput")
    bqv = nc.dram_tensor("bqv", [128, 3], F32, kind="ExternalInput")
    wo = nc.dram_tensor("wo", [128, E], BF16, kind="ExternalInput")
    km = nc.dram_tensor("km", [128, B * KT], F32, kind="ExternalInput")
    ones = nc.dram_tensor("ones", [128, 128], F32R, kind="ExternalInput")
    opart = nc.dram_tensor("opart", [B, S, E], F32, kind="ExternalOutput")

    with tile.TileContext(nc) as tc:
        with (
            tc.tile_pool(name="const", bufs=1) as const,
            tc.tile_pool(name="xp", bufs=3) as xp,
            tc.tile_pool(name="qk", bufs=2) as qk,
            tc.tile_pool(name="vt", bufs=2) as vtp,
            tc.tile_pool(name="vall", bufs=2) as vallp,
            tc.tile_pool(name="ep", bufs=3) as ep,
            tc.tile_pool(name="ctxsb", bufs=3) as ctxsbp,
            tc.tile_pool(name="dnp", bufs=2) as dnp,
            tc.tile_pool(name="outsb", bufs=2) as outsbp,
            tc.tile_pool(name="scps", bufs=2, space="PSUM") as scps,
            tc.tile_pool(name="ctxps", bufs=1, space="PSUM") as ctxps,
            tc.tile_pool(name="auxps", bufs=2, space="PSUM") as auxps,
        ):
            # x segments: (chunk index, col offset within chunk, width);
            # chunk 0 split in half so the first projection starts sooner
            SEGS = [(0, 0, 256), (0, 256, 256), (1, 0, 512), (2, 0, 512),
                    (3, 0, 512)]

            def load_x_seg(b, seg):
                jt, c0, w = seg
                xc = xp.tile([128, IT, w], F32R,
                             tag="x" if w == 512 else "xh",
                             bufs=3 if w == 512 else 2)
                nc.sync.dma_start(out=xc, in_=xT[b, :, jt, :, c0:c0 + w])
                return xc

            # ---- constants (first x segment first: critical path) ----
            x_pre = [load_x_seg(0, SEGS[0])]
            wq_sb = const.tile([128, IT, 128], F32R, tag="wq")
            nc.sync.dma_start(out=wq_sb, in_=wq[:, :, :])
            x_pre.append(load_x_seg(0, SEGS[1]))
            wk_sb = const.tile([128, IT, 128], F32R, tag="wk")
            wv_sb = const.tile([128, IT, 128], F32R, tag="wv")
            nc.sync.dma_start(out=wk_sb, in_=wk[:, :, :])
            nc.sync.dma_start(out=wv_sb, in_=wv[:, :, :])
            bqv_sb = const.tile([128, 3], F32, tag="bqv")
            nc.sync.dma_start(out=bqv_sb, in_=bqv[:, :])
            wo_sb = const.tile([128, E], BF16, tag="wo")
            nc.sync.dma_start(out=wo_sb, in_=wo[:, :])
            km_sb = const.tile([128, B * KT], F32, tag="km")
            nc.sync.dma_start(out=km_sb, in_=km[:, :])
            ones_sb = const.tile([128, 128], F32R, tag="ones")
            nc.sync.dma_start(out=ones_sb, in_=ones[:, :])
            ident = const.tile([128, 128], BF16, tag="ident")
            nc.gpsimd.memset(ident[:], 0.0)
            nc.gpsimd.affine_select(
                out=ident[:], in_=ident[:],
                compare_op=mybir.AluOpType.not_equal, fill=1.0,
                base=0, pattern=[[-1, 128]], channel_multiplier=1,
            )
            # causal triangle for the 128x128 diagonal boundary block:
            # tri[k, q] = 1.0 where q >= k else 0.0
            tri = const.tile([128, 128], BF16, tag="tri")
            nc.gpsimd.memset(tri[:], 1.0)
            nc.gpsimd.affine_select(
                out=tri[:], in_=tri[:],
                compare_op=mybir.AluOpType.is_ge, fill=0.0,
                base=0, pattern=[[1, 128]], channel_multiplier=-1,
            )

            qkv = {}

            def make_proj_chains(b, x_segs):
                """P1 as a list of single-psum-chain closures (spreadable)."""
                qT = qk.tile([128, S], F32R, tag="qT", name=f"qT{b}")
                kT = qk.tile([128, S], F32R, tag="kT", name=f"kT{b}")
                vT = vtp.tile([128, S], BF16, tag="vT", name=f"vT{b}")
                qkv[b] = (qT, kT, vT)
                chains = []
                for si, (jt, c0, w) in enumerate(SEGS):
                    col = jt * 512 + c0
                    for (w_sb, dst, bcol) in ((wq_sb, qT, 0),
                                              (wk_sb, kT, 1),
                                              (wv_sb, vT, 2)):
                        def chain(si=si, w=w, col=col, w_sb=w_sb, dst=dst,
                                  bcol=bcol):
                            ps = auxps.tile([128, 512], F32, tag="aux",
                                            name="ps")
                            for a in range(IT):
                                nc.tensor.matmul(
                                    ps[:, 0:w], w_sb[:, a, :],
                                    x_segs[si][:, a, :],
                                    start=(a == 0), stop=(a == IT - 1),
                                )
                            nc.vector.tensor_scalar_add(
                                out=dst[:, col:col + w], in0=ps[:, 0:w],
                                scalar1=bqv_sb[:, bcol:bcol + 1],
                            )
                        chains.append(chain)
                return chains

            def make_v_tiles(b):
                """P2: v -> natural-layout tiles (ones col = key mask)."""
                vT = qkv[b][2]
                v_all = vallp.tile([128, HPC, KT, 65], BF16, tag="vall",
                                   name=f"vall{b}")
                chains = []
                for ik in range(KT):
                    def chain(ik=ik):
                        tp = auxps.tile([128, 128], BF16, tag="aux",
                                        name="tp")
                        nc.tensor.transpose(
                            tp[:], vT[:, ik * 128:(ik + 1) * 128], ident[:])
                        col = b * KT + ik
                        for h in range(HPC):
                            nc.vector.tensor_scalar_mul(
                                out=v_all[:, h, ik, 0:64],
                                in0=tp[:, h * 64:(h + 1) * 64],
                                scalar1=km_sb[:, col:col + 1],
                            )
                            nc.vector.tensor_copy(
                                out=v_all[:, h, ik, 64:65],
                                in_=km_sb[:, col:col + 1],
                            )
                    chains.append(chain)
                return chains, v_all

            def attention(b, v_all, filler):
                """P3+P4 per jq; pops one filler closure per ik slot."""
                qT, kT, _ = qkv[b]
                for jq in range(QT):
                    nk = 4 * (jq + 1)  # causal: k-tiles 0..nk-1 only
                    ctx = ctxps.tile([65, 1024], F32, tag="ctx", name="ctx")
                    ctxT = ctxsbp.tile([128, 512], BF16, tag="ctxT",
                                       name="ctxT")
                    qsl = slice(jq * 512, (jq + 1) * 512)
                    for ik in range(nk):
                        ksl = slice(ik * 128, (ik + 1) * 128)
                        m = ik - 4 * jq  # >=0 on the diagonal band
                        sc = scps.tile([128, 1024], F32, tag="sc", name="sc")
                        for h in range(HPC):
                            hsl = slice(h * 64, (h + 1) * 64)
                            nc.tensor.matmul(
                                sc[:, h * 512:(h + 1) * 512],
                                kT[hsl, ksl], qT[hsl, qsl],
                                start=True, stop=True,
                            )
                        if m > 0:
                            # fully-masked column block: q < k everywhere
                            nc.vector.memset(
                                sc[:].rearrange("p (h q) -> p h q",
                                                h=2)[:, :, 0:128 * m],
                                -100.0,
                            )
                        e = ep.tile([128, 1024], BF16, tag="e", name="e")
                        nc.scalar.activation(
                            out=e[:], in_=sc[:],
                            func=mybir.ActivationFunctionType.Exp,
                        )
                        if m >= 0:
                            # triangle-mask the 128-wide boundary block
                            for h in range(HPC):
                                bsl = slice(h * 512 + 128 * m,
                                            h * 512 + 128 * m + 128)
                                nc.vector.tensor_mul(
                                    out=e[:, bsl], in0=e[:, bsl], in1=tri[:])
                        for h in range(HPC):
                            nc.tensor.matmul(
                                ctx[:, h * 512:(h + 1) * 512],
                                v_all[:, h, ik, :],
                                e[:, h * 512:(h + 1) * 512],
                                start=(ik == 0), stop=(ik == nk - 1),
                            )
                        if filler:
                            filler.pop(0)()
                    # evacuate ctx psum in one copy (row 64 = denominators),
                    # reciprocal on the denom row, DMA-broadcast, normalize
                    ctxu = dnp.tile([65, 1024], F32R, tag="ctxu", name="ctxu")
                    nc.scalar.copy(out=ctxu[:], in_=ctx[:])
                    for h in range(HPC):
                        hs2 = slice(h * 512, (h + 1) * 512)
                        rb = auxps.tile([128, 512], F32, tag="aux", name="rb")
                        nc.tensor.matmul(
                            rb[:], ones_sb[64:65, :], ctxu[64:65, hs2],
                            start=True, stop=True)
                        rbr = dnp.tile([128, 512], F32, tag="rb", name="rbr")
                        nc.vector.reciprocal_approx_fast(out=rbr[:], in_=rb[:])
                        nc.vector.tensor_mul(
                            out=ctxT[h * 64:(h + 1) * 64, :],
                            in0=ctxu[0:64, hs2].bitcast(F32),
                            in1=rbr[0:64, :],
                        )

                    # out-projection for this jq's 512 tokens (row-sharded)
                    for sub in range(4):
                        jt = jq * 4 + sub
                        tsl = slice(jt * 128, (jt + 1) * 128)
                        osb = outsbp.tile([128, E], F32, tag="osb", name="osb")
                        for half in range(2):
                            osl = slice(half * 512, (half + 1) * 512)
                            po = auxps.tile([128, 512], F32, tag="aux",
                                            name="po")
                            nc.tensor.matmul(
                                po[:], ctxT[:, sub * 128:(sub + 1) * 128],
                                wo_sb[:, osl], start=True, stop=True)
                            if half == 0:
                                nc.scalar.copy(out=osb[:, osl], in_=po[:])
                            else:
                                nc.vector.tensor_copy(out=osb[:, osl],
                                                      in_=po[:])
                        nc.gpsimd.dma_start(out=opart[b, tsl, :], in_=osb[:])

            # ---- emission schedule ----
            x0 = x_pre + [load_x_seg(0, s) for s in SEGS[2:]]
            p1_b0 = make_proj_chains(0, x0)
            for c in p1_b0:
                c()
            p2_b0, vall0 = make_v_tiles(0)
            for c in p2_b0:
                c()
            # b1 P1 chains are spread through b0's attention so the PE has
            # fill work while the exp (ACT) pipeline is the bottleneck
            attention(0, vall0, filler=[])
            x1 = [load_x_seg(1, s) for s in SEGS]
            p1_b1 = make_proj_chains(1, x1)
            for c in p1_b1:
                c()
            p2_b1, vall1 = make_v_tiles(1)
            for c in p2_b1:
                c()
            attention(1, vall1, filler=[])

    nc.compile()
    return nc


_NC = None


def _get_program():
    global _NC
    if _NC is None:
        _NC = _build_program()
    return _NC


def _prep_in_maps(hidden_states, attention_mask, Wq, bq, Wk, bk, Wv, bv, Wo):
    hidden_states = np.asarray(hidden_states, dtype=np.float32)
    attention_mask = np.asarray(attention_mask)
    Wq = np.asarray(Wq, dtype=np.float32)
    Wk = np.asarray(Wk, dtype=np.float32)
    Wv = np.asarray(Wv, dtype=np.float32)
    Wo = np.asarray(Wo, dtype=np.float32)
    bq = np.asarray(bq, dtype=np.float32)
    bk = np.asarray(bk, dtype=np.float32)
    bv = np.asarray(bv, dtype=np.float32)

    # xT[b, p, a, t] = hidden[b, t, 128a+p]  (replicated to every core)
    xT = np.ascontiguousarray(
        hidden_states.transpose(0, 2, 1).reshape(B, IT, 128, QT, 512)
        .transpose(0, 2, 3, 1, 4))

    # km[p, b*KT + ik] = attention_mask[b, 128*ik + p]  (multiplicative 0/1)
    km = (attention_mask.astype(np.float32).reshape(B, KT, 128)
          .transpose(2, 0, 1).reshape(128, B * KT))
    km = np.ascontiguousarray(km)

    in_maps = []
    for c in range(NCORES):
        hs = slice(c * EC, (c + 1) * EC)

        def wprep(W, scale=1.0):
            wt = (scale * W[hs, :]).T  # [E, EC]
            return np.ascontiguousarray(
                wt.reshape(IT, 128, EC).transpose(1, 0, 2))

        bqv = np.stack([SCALE * bq[hs], bk[hs], bv[hs]], axis=1)
        in_maps.append({
            "xT": xT,
            "wq": wprep(Wq, SCALE),
            "wk": wprep(Wk),
            "wv": wprep(Wv),
            "bqv": np.ascontiguousarray(bqv),
            "wo": np.ascontiguousarray(Wo[:, hs].T).astype(ml_dtypes.bfloat16),
            "km": km,
            "ones": np.ones((128, 128), dtype=np.float32),
        })
    return in_maps


def kernel(hidden_states, attention_mask, Wq, bq, Wk, bk, Wv, bv, Wo, bo):
    in_maps = _prep_in_maps(hidden_states, attention_mask,
                            Wq, bq, Wk, bk, Wv, bv, Wo)
    bo = np.asarray(bo, dtype=np.float32)
    nc = _get_program()
    res = run_bass_kernel_spmd(nc, in_maps, core_ids=list(range(NCORES)))

    out = res.results[0]["opart"].astype(np.float64)
    for c in range(1, NCORES):
        out += res.results[c]["opart"]
    out += bo
    return out.astype(np.float32)


# revision 29
# speedup vs baseline: 1.6495x; 1.0127x over previous
"""Diagonal(causal)-masked multi-head attention block on 8 trn2 NeuronCores.

Sharding: tensor-parallel over heads (16 heads -> 2 per core); every core
processes both batch elements for its 2 heads.  q/k/v projections are
column-sharded, out-projection is row-sharded; the partial outputs are
summed on the host (+ output bias).

Per-core dataflow (b in {0,1}, local heads h in {0,1}):
  P1  qT/kT/vT[dim, token] = W.T-chunks @ xT       (f32r, N=512 matmuls)
  P2  v_nat[token, dim] via PE transpose (bf16), ones-column appended
      (ones column carries the key-padding mask -> softmax denominator)
  P3  scores.T[k,q] per (k-tile 128, q-tile 512); both heads packed in one
      [128,1024] PSUM pair-tile (row-group-concurrent matmuls, K=64);
      exp on ACT (no max subtraction -- scores are O(1) by construction);
      causal zeroing via gpsimd affine_select on the bf16 E tile;
      PV: ctx.T[65, q] += v'[k,65].T @ E[k,q]  (bf16), row 64 = denom;
      normalize via K=2 broadcast matmul + DVE muls -> ctxT[128e, t] bf16
  P4  out_partial[t, :] = ctxT-chunk.T @ WoT  (bf16, row-sharded)
"""

import numpy as np
import ml_dtypes

import concourse.bass as bass
import concourse.bacc as bacc
import concourse.mybir as mybir
import concourse.tile as tile
from concourse.bass_utils import run_bass_kernel_spmd

B = 2
S = 2048
E = 1024
H = 16
DH = 64
SCALE = DH**-0.5
NCORES = 8
HPC = H // NCORES  # heads per core (2)
EC = HPC * DH  # embed slice per core (128)

F32 = mybir.dt.float32
F32R = mybir.dt.float32r
BF16 = mybir.dt.bfloat16

KT = S // 128  # 16 k-tiles per sequence
QT = S // 512  # 4 q-tiles per sequence
IT = E // 128  # 8 contraction chunks for projections


def _build_program():
    nc = bacc.Bacc("TRN2", target_bir_lowering=False, debug=False,
                   num_devices=NCORES)

    xT = nc.dram_tensor("xT", [B, 128, QT, IT, 512], F32R,
                        kind="ExternalInput")
    wq = nc.dram_tensor("wq", [128, IT, 128], F32R, kind="ExternalInput")
    wk = nc.dram_tensor("wk", [128, IT, 128], F32R, kind="ExternalInput")
    wv = nc.dram_tensor("wv", [128, IT, 128], F32R, kind="ExternalInput")
    bqv = nc.dram_tensor("bqv", [128, 3], F32, kind="ExternalInput")
    wo = nc.dram_tensor("wo", [128, E], BF16, kind="ExternalInput")
    km = nc.dram_tensor("km", [128, B * KT], F32, kind="ExternalInput")
    ones = nc.dram_tensor("ones", [128, 128], F32R, kind="ExternalInput")
    opart = nc.dram_tensor("opart", [B, S, E], F32, kind="ExternalOutput")

    with tile.TileContext(nc) as tc:
        with (
            tc.tile_pool(name="const", bufs=1) as const,
            tc.tile_pool(name="xp", bufs=3) as xp,
            tc.tile_pool(name="qk", bufs=2) as qk,
            tc.tile_pool(name="vt", bufs=2) as vtp,
            tc.tile_pool(name="vall", bufs=2) as vallp,
            tc.tile_pool(name="ep", bufs=3) as ep,
            tc.tile_pool(name="ctxsb", bufs=3) as ctxsbp,
            tc.tile_pool(name="dnp", bufs=2) as dnp,
            tc.tile_pool(name="outsb", bufs=2) as outsbp,
            tc.tile_pool(name="scps", bufs=2, space="PSUM") as scps,
            tc.tile_pool(name="ctxps", bufs=1, space="PSUM") as ctxps,
            tc.tile_pool(name="auxps", bufs=2, space="PSUM") as auxps,
        ):
            # x segments: (chunk index, col offset within chunk, width);
            # chunk 0 split in half so the first projection starts sooner
            SEGS = [(0, 0, 256), (0, 256, 256), (1, 0, 512), (2, 0, 512),
                    (3, 0, 512)]

            def load_x_seg(b, seg):
                jt, c0, w = seg
                xc = xp.tile([128, IT, w], F32R,
                             tag="x" if w == 512 else "xh",
                             bufs=3 if w == 512 else 2)
                nc.sync.dma_start(out=xc, in_=xT[b, :, jt, :, c0:c0 + w])
                return xc

            # ---- constants (first x segment first: critical path) ----
            x_pre = [load_x_seg(0, SEGS[0])]
            wq_sb = const.tile([128, IT, 128], F32R, tag="wq")
            nc.sync.dma_start(out=wq_sb, in_=wq[:, :, :])
            x_pre.append(load_x_seg(0, SEGS[1]))
            wk_sb = const.tile([128, IT, 128], F32R, tag="wk")
            wv_sb = const.tile([128, IT, 128], F32R, tag="wv")
            nc.sync.dma_start(out=wk_sb, in_=wk[:, :, :])
            nc.sync.dma_start(out=wv_sb, in_=wv[:, :, :])
            bqv_sb = const.tile([128, 3], F32, tag="bqv")
            nc.sync.dma_start(out=bqv_sb, in_=bqv[:, :])
            wo_sb = const.tile([128, E], BF16, tag="wo")
            nc.sync.dma_start(out=wo_sb, in_=wo[:, :])
            km_sb = const.tile([128, B * KT], F32, tag="km")
            nc.sync.dma_start(out=km_sb, in_=km[:, :])
            ones_sb = const.tile([128, 128], F32R, tag="ones")
            nc.sync.dma_start(out=ones_sb, in_=ones[:, :])
            ident = const.tile([128, 128], BF16, tag="ident")
            nc.gpsimd.memset(ident[:], 0.0)
            nc.gpsimd.affine_select(
                out=ident[:], in_=ident[:],
                compare_op=mybir.AluOpType.not_equal, fill=1.0,
                base=0, pattern=[[-1, 128]], channel_multiplier=1,
            )
            # causal triangle for the 128x128 diagonal boundary block:
            # tri[k, q] = 1.0 where q >= k else 0.0
            tri = const.tile([128, 128], BF16, tag="tri")
            nc.gpsimd.memset(tri[:], 1.0)
            nc.gpsimd.affine_select(
                out=tri[:], in_=tri[:],
                compare_op=mybir.AluOpType.is_ge, fill=0.0,
                base=0, pattern=[[1, 128]], channel_multiplier=-1,
            )

            qkv = {}

            def make_proj_chains(b, x_segs):
                """P1 as a list of single-psum-chain closures (spreadable)."""
                qT = qk.tile([128, S], F32R, tag="qT", name=f"qT{b}")
                kT = qk.tile([128, S], F32R, tag="kT", name=f"kT{b}")
                vT = vtp.tile([128, S], BF16, tag="vT", name=f"vT{b}")
                qkv[b] = (qT, kT, vT)
                chains = []
                for si, (jt, c0, w) in enumerate(SEGS):
                    col = jt * 512 + c0
                    for (w_sb, dst, bcol) in ((wq_sb, qT, 0),
                                              (wk_sb, kT, 1),
                                              (wv_sb, vT, 2)):
                        def chain(si=si, w=w, col=col, w_sb=w_sb, dst=dst,
                                  bcol=bcol):
                            ps = auxps.tile([128, 512], F32, tag="aux",
                                            name="ps")
                            for a in range(IT):
                                nc.tensor.matmul(
                                    ps[:, 0:w], w_sb[:, a, :],
                                    x_segs[si][:, a, :],
                                    start=(a == 0), stop=(a == IT - 1),
                                )
                            nc.vector.tensor_scalar_add(
                                out=dst[:, col:col + w], in0=ps[:, 0:w],
                                scalar1=bqv_sb[:, bcol:bcol + 1],
                            )
                        chains.append(chain)
                return chains

            def make_v_tiles(b):
                """P2: v -> natural-layout tiles (ones col = key mask)."""
                vT = qkv[b][2]
                v_all = vallp.tile([128, HPC, KT, 65], BF16, tag="vall",
                                   name=f"vall{b}")
                chains = []
                for ik in range(KT):
                    def chain(ik=ik):
                        tp = auxps.tile([128, 128], BF16, tag="aux",
                                        name="tp")
                        nc.tensor.transpose(
                            tp[:], vT[:, ik * 128:(ik + 1) * 128], ident[:])
                        col = b * KT + ik
                        for h in range(HPC):
                            nc.vector.tensor_scalar_mul(
                                out=v_all[:, h, ik, 0:64],
                                in0=tp[:, h * 64:(h + 1) * 64],
                                scalar1=km_sb[:, col:col + 1],
                            )
                            nc.vector.tensor_copy(
                                out=v_all[:, h, ik, 64:65],
                                in_=km_sb[:, col:col + 1],
                            )
                    chains.append(chain)
                return chains, v_all

            def attention(b, v_all, filler):
                """P3+P4 per jq; pops one filler closure per ik slot."""
                qT, kT, _ = qkv[b]
                for jq in range(QT):
                    nk = 4 * (jq + 1)  # causal: k-tiles 0..nk-1 only
                    ctx = ctxps.tile([65, 1024], F32, tag="ctx", name="ctx")
                    ctxT = ctxsbp.tile([128, 512], BF16, tag="ctxT",
                                       name="ctxT")
                    qsl = slice(jq * 512, (jq + 1) * 512)
                    for ik in range(nk):
                        ksl = slice(ik * 128, (ik + 1) * 128)
                        m = ik - 4 * jq  # >=0 on the diagonal band
                        sc = scps.tile([128, 1024], F32, tag="sc", name="sc")
                        for h in range(HPC):
                            hsl = slice(h * 64, (h + 1) * 64)
                            nc.tensor.matmul(
                                sc[:, h * 512:(h + 1) * 512],
                                kT[hsl, ksl], qT[hsl, qsl],
                                start=True, stop=True,
                            )
                        if m > 0:
                            # fully-masked column block: q < k everywhere
                            nc.vector.memset(
                                sc[:].rearrange("p (h q) -> p h q",
                                                h=2)[:, :, 0:128 * m],
                                -100.0,
                            )
                        e = ep.tile([128, 1024], BF16, tag="e", name="e")
                        nc.scalar.activation(
                            out=e[:], in_=sc[:],
                            func=mybir.ActivationFunctionType.Exp,
                        )
                        if m >= 0:
                            # triangle-mask the 128-wide boundary block
                            for h in range(HPC):
                                bsl = slice(h * 512 + 128 * m,
                                            h * 512 + 128 * m + 128)
                                nc.vector.tensor_mul(
                                    out=e[:, bsl], in0=e[:, bsl], in1=tri[:])
                        for h in range(HPC):
                            nc.tensor.matmul(
                                ctx[:, h * 512:(h + 1) * 512],
                                v_all[:, h, ik, :],
                                e[:, h * 512:(h + 1) * 512],
                                start=(ik == 0), stop=(ik == nk - 1),
                            )
                        if filler:
                            filler.pop(0)()
                    # evacuate ctx psum in one copy (row 64 = denominators),
                    # reciprocal on the denom row, DMA-broadcast, normalize
                    ctxu = dnp.tile([65, 1024], F32R, tag="ctxu", name="ctxu")
                    nc.scalar.copy(out=ctxu[:], in_=ctx[:])
                    for h in range(HPC):
                        hs2 = slice(h * 512, (h + 1) * 512)
                        rb = auxps.tile([128, 512], F32, tag="aux", name="rb")
                        nc.tensor.matmul(
                            rb[:], ones_sb[64:65, :], ctxu[64:65, hs2],
                            start=True, stop=True)
                        rbr = dnp.tile([128, 512], F32, tag="rb", name="rbr")
                        nc.vector.reciprocal_approx_fast(out=rbr[:], in_=rb[:])
                        nc.vector.tensor_mul(
                            out=ctxT[h * 64:(h + 1) * 64, :],
                            in0=ctxu[0:64, hs2].bitcast(F32),
                            in1=rbr[0:64, :],
                        )

                    # out-projection for this jq's 512 tokens (row-sharded)
                    for sub in range(4):
                        jt = jq * 4 + sub
                        tsl = slice(jt * 128, (jt + 1) * 128)
                        osb = outsbp.tile([128, E], F32, tag="osb", name="osb")
                        for half in range(2):
                            osl = slice(half * 512, (half + 1) * 512)
                            po = auxps.tile([128, 512], F32, tag="aux",
                                            name="po")
                            nc.tensor.matmul(
                                po[:], ctxT[:, sub * 128:(sub + 1) * 128],
                                wo_sb[:, osl], start=True, stop=True)
                            if half == 0:
                                nc.scalar.copy(out=osb[:, osl], in_=po[:])
                            else:
                                nc.vector.tensor_copy(out=osb[:, osl],
                                                      in_=po[:])
                        nc.gpsimd.dma_start(out=opart[b, tsl, :], in_=osb[:])

            # ---- emission schedule ----
            x0 = x_pre + [load_x_seg(0, s) for s in SEGS[2:]]
            p1_b0 = make_proj_chains(0, x0)
            for c in p1_b0:
                c()
            p2_b0, vall0 = make_v_tiles(0)
            for c in p2_b0:
                c()
            # b1 P1 chains are spread through b0's attention so the PE has
            # fill work while the exp (ACT) pipeline is the bottleneck
            x1 = [load_x_seg(1, s) for s in SEGS]
            p1_b1 = make_proj_chains(1, x1)
            attention(0, vall0, filler=p1_b1)
            for c in p1_b1:
                c()
            p2_b1, vall1 = make_v_tiles(1)
            for c in p2_b1:
                c()
            attention(1, vall1, filler=[])

    nc.compile()
    return nc


_NC = None


def _get_program():
    global _NC
    if _NC is None:
        _NC = _build_program()
    return _NC


def _prep_in_maps(hidden_states, attention_mask, Wq, bq, Wk, bk, Wv, bv, Wo):
    hidden_states = np.asarray(hidden_states, dtype=np.float32)
    attention_mask = np.asarray(attention_mask)
    Wq = np.asarray(Wq, dtype=np.float32)
    Wk = np.asarray(Wk, dtype=np.float32)
    Wv = np.asarray(Wv, dtype=np.float32)
    Wo = np.asarray(Wo, dtype=np.float32)
    bq = np.asarray(bq, dtype=np.float32)
    bk = np.asarray(bk, dtype=np.float32)
    bv = np.asarray(bv, dtype=np.float32)

    # xT[b, p, a, t] = hidden[b, t, 128a+p]  (replicated to every core)
    xT = np.ascontiguousarray(
        hidden_states.transpose(0, 2, 1).reshape(B, IT, 128, QT, 512)
        .transpose(0, 2, 3, 1, 4))

    # km[p, b*KT + ik] = attention_mask[b, 128*ik + p]  (multiplicative 0/1)
    km = (attention_mask.astype(np.float32).reshape(B, KT, 128)
          .transpose(2, 0, 1).reshape(128, B * KT))
    km = np.ascontiguousarray(km)

    in_maps = []
    for c in range(NCORES):
        hs = slice(c * EC, (c + 1) * EC)

        def wprep(W, scale=1.0):
            wt = (scale * W[hs, :]).T  # [E, EC]
            return np.ascontiguousarray(
                wt.reshape(IT, 128, EC).transpose(1, 0, 2))

        bqv = np.stack([SCALE * bq[hs], bk[hs], bv[hs]], axis=1)
        in_maps.append({
            "xT": xT,
            "wq": wprep(Wq, SCALE),
            "wk": wprep(Wk),
            "wv": wprep(Wv),
            "bqv": np.ascontiguousarray(bqv),
            "wo": np.ascontiguousarray(Wo[:, hs].T).astype(ml_dtypes.bfloat16),
            "km": km,
            "ones": np.ones((128, 128), dtype=np.float32),
        })
    return in_maps


def kernel(hidden_states, attention_mask, Wq, bq, Wk, bk, Wv, bv, Wo, bo):
    in_maps = _prep_in_maps(hidden_states, attention_mask,
                            Wq, bq, Wk, bk, Wv, bv, Wo)
    bo = np.asarray(bo, dtype=np.float32)
    nc = _get_program()
    res = run_bass_kernel_spmd(nc, in_maps, core_ids=list(range(NCORES)))

    out = res.results[0]["opart"].astype(np.float64)
    for c in range(1, NCORES):
        out += res.results[c]["opart"]
    out += bo
    return out.astype(np.float32)


# revision 30
# speedup vs baseline: 1.7557x; 1.0644x over previous
"""Diagonal(causal)-masked multi-head attention block on 8 trn2 NeuronCores.

Sharding: tensor-parallel over heads (16 heads -> 2 per core); every core
processes both batch elements for its 2 heads.  q/k/v projections are
column-sharded, out-projection is row-sharded; the partial outputs are
summed on the host (+ output bias).

Per-core dataflow (b in {0,1}, local heads h in {0,1}):
  P1  qT/kT/vT[dim, token] = W.T-chunks @ xT       (f32r, N=512 matmuls)
  P2  v_nat[token, dim] via PE transpose (bf16), ones-column appended
      (ones column carries the key-padding mask -> softmax denominator)
  P3  scores.T[k,q] per (k-tile 128, q-tile 512); both heads packed in one
      [128,1024] PSUM pair-tile (row-group-concurrent matmuls, K=64);
      exp on ACT (no max subtraction -- scores are O(1) by construction);
      causal zeroing via gpsimd affine_select on the bf16 E tile;
      PV: ctx.T[65, q] += v'[k,65].T @ E[k,q]  (bf16), row 64 = denom;
      normalize via K=2 broadcast matmul + DVE muls -> ctxT[128e, t] bf16
  P4  out_partial[t, :] = ctxT-chunk.T @ WoT  (bf16, row-sharded)
"""

import numpy as np
import ml_dtypes

import concourse.bass as bass
import concourse.bacc as bacc
import concourse.mybir as mybir
import concourse.tile as tile
from concourse.bass_utils import run_bass_kernel_spmd

B = 2
S = 2048
E = 1024
H = 16
DH = 64
SCALE = DH**-0.5
NCORES = 8
HPC = H // NCORES  # heads per core (2)
EC = HPC * DH  # embed slice per core (128)

F32 = mybir.dt.float32
F32R = mybir.dt.float32r
BF16 = mybir.dt.bfloat16

KT = S // 128  # 16 k-tiles per sequence
QT = S // 512  # 4 q-tiles per sequence
IT = E // 128  # 8 contraction chunks for projections


def _build_program():
    nc = bacc.Bacc("TRN2", target_bir_lowering=False, debug=False,
                   num_devices=NCORES)

    xT = nc.dram_tensor("xT", [B, 128, QT, IT, 512], BF16,
                        kind="ExternalInput")
    wq = nc.dram_tensor("wq", [128, IT, 128], BF16, kind="ExternalInput")
    wk = nc.dram_tensor("wk", [128, IT, 128], BF16, kind="ExternalInput")
    wv = nc.dram_tensor("wv", [128, IT, 128], BF16, kind="ExternalInput")
    bqv = nc.dram_tensor("bqv", [128, 3], F32, kind="ExternalInput")
    wo = nc.dram_tensor("wo", [128, E], BF16, kind="ExternalInput")
    km = nc.dram_tensor("km", [128, B * KT], F32, kind="ExternalInput")
    ones = nc.dram_tensor("ones", [128, 128], F32R, kind="ExternalInput")
    opart = nc.dram_tensor("opart", [B, S, E], F32, kind="ExternalOutput")

    with tile.TileContext(nc) as tc:
        with (
            tc.tile_pool(name="const", bufs=1) as const,
            tc.tile_pool(name="xp", bufs=3) as xp,
            tc.tile_pool(name="qk", bufs=2) as qk,
            tc.tile_pool(name="vt", bufs=2) as vtp,
            tc.tile_pool(name="vall", bufs=2) as vallp,
            tc.tile_pool(name="ep", bufs=3) as ep,
            tc.tile_pool(name="ctxsb", bufs=3) as ctxsbp,
            tc.tile_pool(name="dnp", bufs=2) as dnp,
            tc.tile_pool(name="outsb", bufs=2) as outsbp,
            tc.tile_pool(name="scps", bufs=2, space="PSUM") as scps,
            tc.tile_pool(name="ctxps", bufs=1, space="PSUM") as ctxps,
            tc.tile_pool(name="auxps", bufs=2, space="PSUM") as auxps,
        ):
            # x segments: (chunk index, col offset within chunk, width);
            # chunk 0 split in half so the first projection starts sooner
            SEGS = [(0, 0, 256), (0, 256, 256), (1, 0, 512), (2, 0, 512),
                    (3, 0, 512)]

            def load_x_seg(b, seg):
                jt, c0, w = seg
                xc = xp.tile([128, IT, w], BF16,
                             tag="x" if w == 512 else "xh",
                             bufs=3 if w == 512 else 2)
                nc.sync.dma_start(out=xc, in_=xT[b, :, jt, :, c0:c0 + w])
                return xc

            # ---- constants (first x segment first: critical path) ----
            x_pre = [load_x_seg(0, SEGS[0])]
            wq_sb = const.tile([128, IT, 128], BF16, tag="wq")
            nc.sync.dma_start(out=wq_sb, in_=wq[:, :, :])
            x_pre.append(load_x_seg(0, SEGS[1]))
            wk_sb = const.tile([128, IT, 128], BF16, tag="wk")
            wv_sb = const.tile([128, IT, 128], BF16, tag="wv")
            nc.sync.dma_start(out=wk_sb, in_=wk[:, :, :])
            nc.sync.dma_start(out=wv_sb, in_=wv[:, :, :])
            bqv_sb = const.tile([128, 3], F32, tag="bqv")
            nc.sync.dma_start(out=bqv_sb, in_=bqv[:, :])
            wo_sb = const.tile([128, E], BF16, tag="wo")
            nc.sync.dma_start(out=wo_sb, in_=wo[:, :])
            km_sb = const.tile([128, B * KT], F32, tag="km")
            nc.sync.dma_start(out=km_sb, in_=km[:, :])
            ones_sb = const.tile([128, 128], F32R, tag="ones")
            nc.sync.dma_start(out=ones_sb, in_=ones[:, :])
            ident = const.tile([128, 128], BF16, tag="ident")
            nc.gpsimd.memset(ident[:], 0.0)
            nc.gpsimd.affine_select(
                out=ident[:], in_=ident[:],
                compare_op=mybir.AluOpType.not_equal, fill=1.0,
                base=0, pattern=[[-1, 128]], channel_multiplier=1,
            )
            # causal triangle for the 128x128 diagonal boundary block:
            # tri[k, q] = 1.0 where q >= k else 0.0
            tri = const.tile([128, 128], BF16, tag="tri")
            nc.gpsimd.memset(tri[:], 1.0)
            nc.gpsimd.affine_select(
                out=tri[:], in_=tri[:],
                compare_op=mybir.AluOpType.is_ge, fill=0.0,
                base=0, pattern=[[1, 128]], channel_multiplier=-1,
            )

            qkv = {}

            def make_proj_chains(b, x_segs):
                """P1 as a list of single-psum-chain closures (spreadable)."""
                qT = qk.tile([128, S], F32R, tag="qT", name=f"qT{b}")
                kT = qk.tile([128, S], F32R, tag="kT", name=f"kT{b}")
                vT = vtp.tile([128, S], BF16, tag="vT", name=f"vT{b}")
                qkv[b] = (qT, kT, vT)
                chains = []
                for si, (jt, c0, w) in enumerate(SEGS):
                    col = jt * 512 + c0
                    for (w_sb, dst, bcol) in ((wq_sb, qT, 0),
                                              (wk_sb, kT, 1),
                                              (wv_sb, vT, 2)):
                        def chain(si=si, w=w, col=col, w_sb=w_sb, dst=dst,
                                  bcol=bcol):
                            ps = auxps.tile([128, 512], F32, tag="aux",
                                            name="ps")
                            for a in range(IT):
                                nc.tensor.matmul(
                                    ps[:, 0:w], w_sb[:, a, :],
                                    x_segs[si][:, a, :],
                                    start=(a == 0), stop=(a == IT - 1),
                                )
                            nc.vector.tensor_scalar_add(
                                out=dst[:, col:col + w], in0=ps[:, 0:w],
                                scalar1=bqv_sb[:, bcol:bcol + 1],
                            )
                        chains.append(chain)
                return chains

            def make_v_tiles(b):
                """P2: v -> natural-layout tiles (ones col = key mask)."""
                vT = qkv[b][2]
                v_all = vallp.tile([128, HPC, KT, 65], BF16, tag="vall",
                                   name=f"vall{b}")
                chains = []
                for ik in range(KT):
                    def chain(ik=ik):
                        tp = auxps.tile([128, 128], BF16, tag="aux",
                                        name="tp")
                        nc.tensor.transpose(
                            tp[:], vT[:, ik * 128:(ik + 1) * 128], ident[:])
                        col = b * KT + ik
                        for h in range(HPC):
                            nc.vector.tensor_scalar_mul(
                                out=v_all[:, h, ik, 0:64],
                                in0=tp[:, h * 64:(h + 1) * 64],
                                scalar1=km_sb[:, col:col + 1],
                            )
                            nc.vector.tensor_copy(
                                out=v_all[:, h, ik, 64:65],
                                in_=km_sb[:, col:col + 1],
                            )
                    chains.append(chain)
                return chains, v_all

            def attention(b, v_all, filler, jq_order=(0, 1, 2, 3)):
                """P3+P4 per jq; pops one filler closure per ik slot."""
                qT, kT, _ = qkv[b]
                for jq in jq_order:
                    nk = 4 * (jq + 1)  # causal: k-tiles 0..nk-1 only
                    ctx = ctxps.tile([65, 1024], F32, tag="ctx", name="ctx")
                    ctxT = ctxsbp.tile([128, 512], BF16, tag="ctxT",
                                       name="ctxT")
                    qsl = slice(jq * 512, (jq + 1) * 512)
                    for ik in range(nk):
                        ksl = slice(ik * 128, (ik + 1) * 128)
                        m = ik - 4 * jq  # >=0 on the diagonal band
                        sc = scps.tile([128, 1024], F32, tag="sc", name="sc")
                        for h in range(HPC):
                            hsl = slice(h * 64, (h + 1) * 64)
                            nc.tensor.matmul(
                                sc[:, h * 512:(h + 1) * 512],
                                kT[hsl, ksl], qT[hsl, qsl],
                                start=True, stop=True,
                            )
                        if m > 0:
                            # fully-masked column block: q < k everywhere
                            nc.vector.memset(
                                sc[:].rearrange("p (h q) -> p h q",
                                                h=2)[:, :, 0:128 * m],
                                -100.0,
                            )
                        e = ep.tile([128, 1024], BF16, tag="e", name="e")
                        nc.scalar.activation(
                            out=e[:], in_=sc[:],
                            func=mybir.ActivationFunctionType.Exp,
                        )
                        if m >= 0:
                            # triangle-mask the 128-wide boundary block
                            for h in range(HPC):
                                bsl = slice(h * 512 + 128 * m,
                                            h * 512 + 128 * m + 128)
                                nc.vector.tensor_mul(
                                    out=e[:, bsl], in0=e[:, bsl], in1=tri[:])
                        for h in range(HPC):
                            nc.tensor.matmul(
                                ctx[:, h * 512:(h + 1) * 512],
                                v_all[:, h, ik, :],
                                e[:, h * 512:(h + 1) * 512],
                                start=(ik == 0), stop=(ik == nk - 1),
                            )
                        if filler:
                            filler.pop(0)()
                    # evacuate ctx psum in one copy (row 64 = denominators),
                    # reciprocal on the denom row, DMA-broadcast, normalize
                    ctxu = dnp.tile([65, 1024], F32R, tag="ctxu", name="ctxu")
                    nc.scalar.copy(out=ctxu[:], in_=ctx[:])
                    for h in range(HPC):
                        hs2 = slice(h * 512, (h + 1) * 512)
                        rb = auxps.tile([128, 512], F32, tag="aux", name="rb")
                        nc.tensor.matmul(
                            rb[:], ones_sb[64:65, :], ctxu[64:65, hs2],
                            start=True, stop=True)
                        rbr = dnp.tile([128, 512], F32, tag="rb", name="rbr")
                        nc.vector.reciprocal_approx_fast(out=rbr[:], in_=rb[:])
                        nc.vector.tensor_mul(
                            out=ctxT[h * 64:(h + 1) * 64, :],
                            in0=ctxu[0:64, hs2].bitcast(F32),
                            in1=rbr[0:64, :],
                        )

                    # out-projection for this jq's 512 tokens (row-sharded)
                    for sub in range(4):
                        jt = jq * 4 + sub
                        tsl = slice(jt * 128, (jt + 1) * 128)
                        osb = outsbp.tile([128, E], F32, tag="osb", name="osb")
                        for half in range(2):
                            osl = slice(half * 512, (half + 1) * 512)
                            po = auxps.tile([128, 512], F32, tag="aux",
                                            name="po")
                            nc.tensor.matmul(
                                po[:], ctxT[:, sub * 128:(sub + 1) * 128],
                                wo_sb[:, osl], start=True, stop=True)
                            if half == 0:
                                nc.scalar.copy(out=osb[:, osl], in_=po[:])
                            else:
                                nc.vector.tensor_copy(out=osb[:, osl],
                                                      in_=po[:])
                        nc.gpsimd.dma_start(out=opart[b, tsl, :], in_=osb[:])

            # ---- emission schedule ----
            x0 = x_pre + [load_x_seg(0, s) for s in SEGS[2:]]
            p1_b0 = make_proj_chains(0, x0)
            for c in p1_b0:
                c()
            p2_b0, vall0 = make_v_tiles(0)
            for c in p2_b0:
                c()
            # b1 P1 chains are spread through b0's attention so the PE has
            # fill work while the exp (ACT) pipeline is the bottleneck
            x1 = [load_x_seg(1, s) for s in SEGS]
            p1_b1 = make_proj_chains(1, x1)
            p2_b1, vall1 = make_v_tiles(1)
            fill = p1_b1 + p2_b1
            attention(0, vall0, filler=fill)
            for c in fill:
                c()
            attention(1, vall1, filler=[], jq_order=(3, 2, 1, 0))

    nc.compile()
    return nc


_NC = None


def _get_program():
    global _NC
    if _NC is None:
        _NC = _build_program()
    return _NC


def _prep_in_maps(hidden_states, attention_mask, Wq, bq, Wk, bk, Wv, bv, Wo):
    hidden_states = np.asarray(hidden_states, dtype=np.float32)
    attention_mask = np.asarray(attention_mask)
    Wq = np.asarray(Wq, dtype=np.float32)
    Wk = np.asarray(Wk, dtype=np.float32)
    Wv = np.asarray(Wv, dtype=np.float32)
    Wo = np.asarray(Wo, dtype=np.float32)
    bq = np.asarray(bq, dtype=np.float32)
    bk = np.asarray(bk, dtype=np.float32)
    bv = np.asarray(bv, dtype=np.float32)

    # xT[b, p, a, t] = hidden[b, t, 128a+p]  (replicated to every core)
    xT = np.ascontiguousarray(
        hidden_states.transpose(0, 2, 1).reshape(B, IT, 128, QT, 512)
        .transpose(0, 2, 3, 1, 4)).astype(ml_dtypes.bfloat16)

    # km[p, b*KT + ik] = attention_mask[b, 128*ik + p]  (multiplicative 0/1)
    km = (attention_mask.astype(np.float32).reshape(B, KT, 128)
          .transpose(2, 0, 1).reshape(128, B * KT))
    km = np.ascontiguousarray(km)

    in_maps = []
    for c in range(NCORES):
        hs = slice(c * EC, (c + 1) * EC)

        def wprep(W, scale=1.0):
            wt = (scale * W[hs, :]).T  # [E, EC]
            return np.ascontiguousarray(
                wt.reshape(IT, 128, EC).transpose(1, 0, 2)
            ).astype(ml_dtypes.bfloat16)

        bqv = np.stack([SCALE * bq[hs], bk[hs], bv[hs]], axis=1)
        in_maps.append({
            "xT": xT,
            "wq": wprep(Wq, SCALE),
            "wk": wprep(Wk),
            "wv": wprep(Wv),
            "bqv": np.ascontiguousarray(bqv),
            "wo": np.ascontiguousarray(Wo[:, hs].T).astype(ml_dtypes.bfloat16),
            "km": km,
            "ones": np.ones((128, 128), dtype=np.float32),
        })
    return in_maps


def kernel(hidden_states, attention_mask, Wq, bq, Wk, bk, Wv, bv, Wo, bo):
    in_maps = _prep_in_maps(hidden_states, attention_mask,
                            Wq, bq, Wk, bk, Wv, bv, Wo)
    bo = np.asarray(bo, dtype=np.float32)
    nc = _get_program()
    res = run_bass_kernel_spmd(nc, in_maps, core_ids=list(range(NCORES)))

    out = res.results[0]["opart"].astype(np.float64)
    for c in range(1, NCORES):
        out += res.results[c]["opart"]
    out += bo
    return out.astype(np.float32)
